# revision 1
# baseline (speedup 1.0000x reference)
#!/usr/bin/env python3
"""Bass/Trainium2 kernel for nn_DiffusionTransformer_17738214932657.

8-core SPMD. Sharding:
  - attention rows (sequence) sharded: core c owns seq rows [c*96, (c+1)*96)
  - pair bias: row-sharded (local to attention rows) -> no pair comm
  - qkv/gate/out weights: replicated reads, k/v computed full, q/g for own rows
  - transition tr1/tr_out: column-sharded (192+192 cols, padded to 256+256)
  - cond (AdaLN) logits: column-sharded compute + one AllToAll -> row shards
  - per block collectives: AllGather(x^T), AllGather(xt^T), ReduceScatter(u2)
Activations on-chip are channel-major ("transposed"): (ch-part, seq-free).
All matmuls bf16 with fp32 PSUM accumulation; residual kept fp32 in SBUF.
"""
import numpy as np

P = 8
NB, SB, NH = 24, 4, 16
CA, CS, CP = 768, 384, 128
N = 768
DH = 48
NS = N // P            # 96 seq rows per core
NSB = NB // SB         # 6 superblocks
NCH = CA // 128        # 6 ch tiles
TRC = 512              # padded per-core tr1 cols (192a+64z+192b+64z)
TRK = 256              # padded per-core tr_out contraction rows
CAP = 1024             # padded head space: head h at 64*h
NCP = CAP // 128       # 8 padded ch tiles
SCALE = DH ** -0.5
EPS = 1e-5

_CACHE = {}


# ----------------------------------------------------------------- host prep
def _f32(x):
    return np.ascontiguousarray(np.asarray(x), dtype=np.float32)


def _bf16(x):
    import ml_dtypes
    return np.ascontiguousarray(
        np.asarray(x, dtype=np.float32).astype(ml_dtypes.bfloat16))


def _ln_rows(x, eps=EPS):
    m = x.mean(-1, keepdims=True)
    v = ((x - m) ** 2).mean(-1, keepdims=True)
    return (x - m) / np.sqrt(v + eps)


def host_prep(inputs):
    """Returns list of 8 per-core input dicts."""
    act = _f32(inputs["act"])                   # (N, CA)
    mask = _f32(inputs["mask"])                 # (N,)
    sc = _f32(inputs["single_cond"])            # (N, CS)
    pc = np.asarray(inputs["pair_cond"])        # (N, N, CP) f32
    plw = _f32(inputs["pair_ln_w"])             # (CP,)
    plb = _f32(inputs["pair_ln_b"])
    ppw = _f32(inputs["pair_proj_w"])           # (NSB, CP, 64)
    ppb = _f32(inputs["pair_proj_b"])           # (NSB, 64)

    nscT = _bf16(_ln_rows(sc).T)                # (CS, N)

    # fold pair LN weight/bias into proj
    ppw_f = ppw * plw[None, :, None]
    ppb_f = ppb + np.einsum("c,sch->sh", plb, ppw)

    sa_cln = _f32(inputs["sa_cln_w"])           # (NB, CS)
    tr_cln = _f32(inputs["tr_cln_w"])
    mats = []
    for nm, cln in [("sa_scale_w", sa_cln), ("sa_bias_w", sa_cln),
                    ("sa_azc_w", sa_cln), ("tr_scale_w", tr_cln),
                    ("tr_bias_w", tr_cln), ("tr_azc_w", tr_cln)]:
        mats.append(_f32(inputs[nm]) * cln[:, :, None])
    condw = np.stack(mats, axis=1)              # (NB, 6, CS, CA)
    condb = np.stack([_f32(inputs["sa_scale_b"]), _f32(inputs["sa_azc_b"]),
                      _f32(inputs["tr_scale_b"]), _f32(inputs["tr_azc_b"])],
                     axis=1)                    # (NB, 4, CA)

    wtr1 = _f32(inputs["tr1_w"])                # (NB, CA, 4CA)
    btr1 = _f32(inputs["tr1_b"])                # (NB, 4CA)
    wtr2 = _f32(inputs["tr_out_w"])             # (NB, 2CA, CA)

    # pad head dim: head h (48 ch) -> offset 64*h in a 1024-wide space
    def pad_cols(w):                            # (NB, CA, CA)->(NB, CA, CAP)
        out = np.zeros((NB, w.shape[1], CAP), np.float32)
        for h in range(NH):
            out[:, :, 64 * h:64 * h + DH] = w[:, :, DH * h:DH * (h + 1)]
        return out

    def pad_rows(w):                            # (NB, CA, CA)->(NB, CAP, CA)
        out = np.zeros((NB, CAP, w.shape[2]), np.float32)
        for h in range(NH):
            out[:, 64 * h:64 * h + DH, :] = w[:, DH * h:DH * (h + 1), :]
        return out

    qb_pad = np.zeros((NB, CAP), np.float32)
    for h in range(NH):
        qb_pad[:, 64 * h:64 * h + DH] =             np.asarray(inputs["q_b"], np.float32)[:, DH * h:DH * (h + 1)]

    shared = {
        "mask_in": _f32(mask[:, None]),
        "nscT_in": nscT,
        "wq_in": _bf16(pad_cols(_f32(inputs["q_w"]))),
        "wk_in": _bf16(pad_cols(_f32(inputs["k_w"]))),
        "wv_in": _bf16(inputs["v_w"]),
        "wg_in": _bf16(pad_cols(_f32(inputs["gate_w"]))),
        "qb_in": _f32(qb_pad[:, :, None]),
        "wout_in": _bf16(pad_rows(_f32(inputs["sa_out_w"]))),
        "pairw_in": _bf16(ppw_f),
        "pairb_in": _f32(ppb_f[:, :, None]),
    }

    per_core = []
    for c in range(P):
        rows = slice(c * NS, (c + 1) * NS)
        a_lo, a_hi = c * 192, (c + 1) * 192
        wtr1_c = np.zeros((NB, CA, TRC), np.float32)
        wtr1_c[:, :, 0:192] = wtr1[:, :, a_lo:a_hi]
        wtr1_c[:, :, 256:448] = wtr1[:, :, 2 * CA + a_lo:2 * CA + a_hi]
        btr1_c = np.zeros((NB, TRC), np.float32)
        btr1_c[:, 0:192] = btr1[:, a_lo:a_hi]
        btr1_c[:, 256:448] = btr1[:, 2 * CA + a_lo:2 * CA + a_hi]
        wtr2_c = np.zeros((NB, TRK, CA), np.float32)
        wtr2_c[:, 0:192, :] = wtr2[:, a_lo:a_hi, :]

        d = dict(shared)
        d.update({
            "actT_in": _f32(act[rows].T),
            "condw_in": _bf16(condw[:, :, :, c * NS:(c + 1) * NS]),
            "condb_in": _f32(condb[:, :, c * NS:(c + 1) * NS][:, :, :, None]),
            "wtr1_in": _bf16(wtr1_c),
            "btr1_in": _f32(btr1_c[:, :, None]),
            "wtr2_in": _bf16(wtr2_c),
            "pair_in": _f32(pc[rows]),
        })
        per_core.append(d)
    return per_core


# -------------------------------------------------------------- device build
def build_nc(nblocks=NB, taps=()):
    import sys
    if "/opt/trn_rl_repo" not in sys.path:
        sys.path.insert(0, "/opt/trn_rl_repo")
    import concourse.bacc as bacc
    import concourse.mybir as mybir
    from concourse import tile
    from concourse.alu_op_type import AluOpType as ALU

    F32 = mybir.dt.float32
    BF16 = mybir.dt.bfloat16
    ACTF = mybir.ActivationFunctionType
    RG = [list(range(P))]

    nc = bacc.Bacc("TRN2", target_bir_lowering=False, debug=False,
                   num_devices=P)

    def din(name, shape, dt=F32):
        return nc.dram_tensor(name, list(shape), dt, kind="ExternalInput")

    actT_in = din("actT_in", (CA, NS))
    mask_in = din("mask_in", (N, 1))
    nscT_in = din("nscT_in", (CS, N), BF16)
    condw_in = din("condw_in", (NB, 6, CS, NS), BF16)
    condb_in = din("condb_in", (NB, 4, NS, 1))
    wq_in = din("wq_in", (NB, CA, CAP), BF16)
    wk_in = din("wk_in", (NB, CA, CAP), BF16)
    wv_in = din("wv_in", (NB, CA, CA), BF16)
    wg_in = din("wg_in", (NB, CA, CAP), BF16)
    qb_in = din("qb_in", (NB, CAP, 1))
    wout_in = din("wout_in", (NB, CAP, CA), BF16)
    wtr1_in = din("wtr1_in", (NB, CA, TRC), BF16)
    btr1_in = din("btr1_in", (NB, TRC, 1))
    wtr2_in = din("wtr2_in", (NB, TRK, CA), BF16)
    pair_in = din("pair_in", (NS, N, CP))
    pairw_in = din("pairw_in", (NSB, CP, 64), BF16)
    pairb_in = din("pairb_in", (NSB, 64, 1))

    outT = nc.dram_tensor("outT", [CA, NS], F32, kind="ExternalOutput")

    tap_outs = {}

    # ---- internal DRAM ----
    a2a_in = nc.dram_tensor("a2a_in", [P, nblocks, 6, NS, NS], BF16)
    a2a_out = nc.dram_tensor("a2a_out", [P, nblocks, 6, NS, NS], BF16)
    pa_dram = nc.dram_tensor("pa_dram", [N * NS, CP], BF16)
    pl_dram = nc.dram_tensor("pl_dram", [NSB, 64, N, NS], BF16)
    agx_in = [nc.dram_tensor(f"agx_in{i}", [CA, NS], BF16)
              for i in range(nblocks)]
    agx_out = [nc.dram_tensor(f"agx_out{i}", [P * CA, NS], BF16,
                              addr_space="Shared") for i in range(nblocks)]
    agt_in = [nc.dram_tensor(f"agt_in{i}", [CA, NS], BF16)
              for i in range(nblocks)]
    agt_out = [nc.dram_tensor(f"agt_out{i}", [P * CA, NS], BF16,
                              addr_space="Shared") for i in range(nblocks)]
    rs_in = [nc.dram_tensor(f"rs_in{i}", [P, CA, NS], BF16)
             for i in range(nblocks)]
    rs_out = [nc.dram_tensor(f"rs_out{i}", [CA, NS], BF16)
              for i in range(nblocks)]

    with tile.TileContext(nc, num_cores=P) as tc:
        with (
            tc.tile_pool(name="persist", bufs=1) as pp,
            tc.tile_pool(name="condp", bufs=2) as cpool,
            tc.tile_pool(name="plp", bufs=2) as plpool,
            tc.tile_pool(name="scr", bufs=4) as scr,
            tc.tile_pool(name="scr1", bufs=3) as scr1,
            tc.tile_pool(name="pairp", bufs=3) as pairp,
            tc.tile_pool(name="ps", bufs=8, space="PSUM") as ps,
        ):
            # ---------------- persistent tiles ----------------
            actT = pp.tile([128, NCH * NS], F32, tag="actT")
            ones = pp.tile([128, 1], F32, tag="ones")
            nc.vector.memset(ones[:], 1.0)
            ones_row = pp.tile([1, 128], F32, tag="ones_row")
            nc.vector.memset(ones_row[:], 1.0)
            epsc = pp.tile([128, 1], F32, tag="epsc")
            nc.vector.memset(epsc[:], EPS)
            mask_sb = pp.tile([128, NCH], F32, tag="mask")
            mask_bf = pp.tile([128, NCH], BF16, tag="maskbf")
            nc.sync.dma_start(
                out=mask_sb[:].rearrange("p (t o) -> p t o", o=1),
                in_=mask_in.ap().rearrange("(t p) o -> p t o", p=128))
            nc.vector.tensor_copy(mask_bf[:], mask_sb[:])
            nscT_sb = pp.tile([128, 3 * N], BF16, tag="nsc")
            nc.sync.dma_start(
                out=nscT_sb[:].rearrange("p (t n) -> p t n", n=N),
                in_=nscT_in.ap().rearrange("(t p) n -> p t n", p=128))
            nc.sync.dma_start(
                out=actT[:].rearrange("p (t s) -> p t s", s=NS),
                in_=actT_in.ap().rearrange("(t p) s -> p t s", p=128))

            X = pp.tile([128, NCH * N], BF16, tag="X", bufs=2)
            kT = pp.tile([128, NCP * N], BF16, tag="kT")         # pad-head
            qT = pp.tile([128, NCP * NS], BF16, tag="qT")        # pad-head
            gsig = pp.tile([128, NCP * NS], BF16, tag="gsig")
            vm = pp.tile([128, NCH * N], BF16, tag="vm")
            expb = pp.tile([128, NCH * (8 * NS)], BF16, tag="expb")
            waT = pp.tile([128, NCP * NS], BF16, tag="waT")
            nc.vector.memset(waT[:], 0.0)
            rcp = pp.tile([1, NH * NS], F32, tag="rcp")
            bca = pp.tile([48, NH * NS], F32, tag="bca")

            # =================================================================
            # cond precompute (col-sharded) + AllToAll
            # =================================================================
            for i in range(nblocks):
                for m in range(6):
                    pst = [ps.tile([NS, 384], F32, tag="ps", name=f"pst{i}_{m}_{x}") for x in range(2)]
                    for ch in range(3):
                        lhs = scr.tile([128, NS], BF16, tag="condw")
                        nc.sync.dma_start(
                            out=lhs[:],
                            in_=condw_in[i, m, 128 * ch:128 * (ch + 1), :])
                        for half in range(2):
                            nc.tensor.matmul(
                                pst[half][:], lhs[:],
                                nscT_sb[:, ch * N + half * 384:
                                        ch * N + (half + 1) * 384],
                                start=(ch == 0), stop=(ch == 2))
                    cnd = scr.tile([NS, N], BF16, tag="condsb")
                    bidx = {0: 0, 2: 1, 3: 2, 5: 3}.get(m)
                    if bidx is not None:
                        bap = scr.tile([NS, 1], F32, tag="condb")
                        nc.sync.dma_start(out=bap[:], in_=condb_in[i, bidx])
                    for half in range(2):
                        osl = cnd[:, half * 384:(half + 1) * 384]
                        if bidx is not None:
                            nc.scalar.activation(osl, pst[half][:],
                                                 ACTF.Sigmoid, bias=bap[:])
                        else:
                            nc.scalar.activation(osl, pst[half][:], ACTF.Copy)
                    nc.sync.dma_start(
                        out=a2a_in[:, i, m, :, :].rearrange("r c s -> c r s"),
                        in_=cnd[:].rearrange("c (r s) -> c r s", r=P))
            nc.gpsimd.collective_compute(
                "AllToAll", mybir.AluOpType.bypass, replica_groups=RG,
                ins=[a2a_in.ap().opt()], outs=[a2a_out.ap().opt()])

            def load_cond(i, m, tag):
                t = cpool.tile([128, NCH * NS], BF16, tag=tag)
                for cht in range(NCH):
                    base = cht * 128
                    while base < (cht + 1) * 128:
                        r = base // NS
                        k0 = base - r * NS
                        cnt = min((r + 1) * NS, (cht + 1) * 128) - base
                        nc.sync.dma_start(
                            out=t[base - cht * 128:base - cht * 128 + cnt,
                                  cht * NS:(cht + 1) * NS],
                            in_=a2a_out[r, i, m, k0:k0 + cnt, :])
                        base += cnt
                return t

            # =================================================================
            # pair precompute
            # =================================================================
            for n in range(NS):
                for jg in range(2):
                    t_in = pairp.tile([128, 3 * CP], F32, tag="pln_in")
                    nc.sync.dma_start(
                        out=t_in[:].rearrange("p (j c) -> p j c", c=CP),
                        in_=pair_in[n, jg * 384:(jg + 1) * 384, :]
                            .rearrange("(j p) c -> p j c", p=128))
                    st6 = pairp.tile([128, 3 * 6], F32, tag="pln_st")
                    for jj in range(3):
                        nc.vector.bn_stats(
                            st6[:, 6 * jj:6 * jj + 6],
                            t_in[:, jj * CP:(jj + 1) * CP])
                    agg = pairp.tile([128, 3 * 2], F32, tag="pln_agg")
                    for jj in range(3):
                        nc.vector.bn_aggr(
                            agg[:, 2 * jj:2 * jj + 2],
                            st6[:, 6 * jj:6 * jj + 6])
                    agv = agg[:].rearrange("p (j k) -> p j k", k=2)
                    sd = pairp.tile([128, 3], F32, tag="pln_sd")
                    nc.scalar.activation(
                        sd[:].rearrange("p (j o) -> p j o", o=1),
                        agv[:, :, 1:2], ACTF.Sqrt, bias=epsc[:])
                    rsq = pairp.tile([128, 3], F32, tag="pln_rs")
                    nc.vector.reciprocal(rsq[:], sd[:])
                    t_out = pairp.tile([128, 3 * CP], BF16, tag="pln_out")
                    for j in range(3):
                        nc.vector.tensor_scalar(
                            t_out[:, j * CP:(j + 1) * CP],
                            t_in[:, j * CP:(j + 1) * CP],
                            agg[:, 2 * j:2 * j + 1], rsq[:, j:j + 1],
                            ALU.subtract, ALU.mult)
                    nc.sync.dma_start(
                        out=pa_dram.ap()
                            .rearrange("(x s n) c -> x n s c", x=2, n=NS)[jg, n]
                            .rearrange("(j p) c -> p j c", p=128),
                        in_=t_out[:].rearrange("p (j c) -> p j c", c=CP))
            NCHK = (N * NS) // 512
            for chk in range(NCHK):
                rhs = pairp.tile([128, 512], BF16, tag="prj_rhs")
                nc.sync.dma_start_transpose(
                    out=rhs[:], in_=pa_dram[chk * 512:(chk + 1) * 512, :])
                for sbx in range(NSB):
                    lw = pairp.tile([128, 64], BF16, tag="prj_w")
                    nc.sync.dma_start(out=lw[:], in_=pairw_in[sbx])
                    pp_ps = ps.tile([64, 512], F32, tag="ps")
                    nc.tensor.matmul(pp_ps[:], lw[:], rhs[:], start=True,
                                     stop=True)
                    bp = pairp.tile([64, 1], F32, tag="prj_b")
                    nc.sync.dma_start(out=bp[:], in_=pairb_in[sbx])
                    ev = pairp.tile([64, 512], BF16, tag="prj_ev")
                    nc.scalar.activation(ev[:], pp_ps[:], ACTF.Identity,
                                         bias=bp[:])
                    nc.sync.dma_start(
                        out=pl_dram[sbx].rearrange("h s n -> h (s n)")
                            [:, chk * 512:(chk + 1) * 512],
                        in_=ev[:])

            # =================================================================
            # block loop helpers
            # =================================================================
            def ln_chmajor(src_f32, dst_bf16):
                s_ps = ps.tile([1, NS], F32, tag="ps")
                q_ps = ps.tile([1, NS], F32, tag="ps")
                sq = scr.tile([128, NS], F32, tag="lnsq")
                for cht in range(NCH):
                    nc.tensor.matmul(s_ps[:], ones[:],
                                     src_f32[:, cht * NS:(cht + 1) * NS],
                                     start=(cht == 0), stop=(cht == NCH - 1))
                for cht in range(NCH):
                    nc.scalar.activation(sq[:],
                                         src_f32[:, cht * NS:(cht + 1) * NS],
                                         ACTF.Square)
                    nc.tensor.matmul(q_ps[:], ones[:], sq[:],
                                     start=(cht == 0), stop=(cht == NCH - 1))
                mr = scr.tile([1, 2 * NS], F32, tag="lnmr")
                nc.vector.tensor_scalar(mr[:, 0:NS], s_ps[:], 1.0 / CA, None,
                                        ALU.mult)
                m2 = scr.tile([1, NS], F32, tag="lnm2")
                nc.vector.tensor_tensor(m2[:], mr[:, 0:NS], mr[:, 0:NS],
                                        ALU.mult)
                nc.vector.tensor_scalar(mr[:, NS:], q_ps[:], 1.0 / CA, None,
                                        ALU.mult)
                nc.vector.tensor_tensor(mr[:, NS:], mr[:, NS:], m2[:],
                                        ALU.subtract)
                sdv = scr.tile([1, NS], F32, tag="lnsd")
                nc.scalar.activation(sdv[:], mr[:, NS:], ACTF.Sqrt,
                                     bias=epsc[0:1, :])
                nc.vector.reciprocal(mr[:, NS:], sdv[:])
                b_ps = ps.tile([128, 2 * NS], F32, tag="ps")
                nc.tensor.matmul(b_ps[:], ones_row[:], mr[:], start=True,
                                 stop=True)
                mb = scr.tile([128, 2 * NS], F32, tag="lnmb")
                nc.vector.tensor_copy(mb[:], b_ps[:])
                for cht in range(NCH):
                    sl = src_f32[:, cht * NS:(cht + 1) * NS]
                    t1 = scr.tile([128, NS], F32, tag="lnt1")
                    nc.vector.tensor_tensor(t1[:], sl, mb[:, 0:NS],
                                            ALU.subtract)
                    nc.vector.tensor_tensor(
                        dst_bf16[:, cht * NS:(cht + 1) * NS], t1[:],
                        mb[:, NS:], ALU.mult)

            def big_matmul(win, i, rhs_tile, rhs_n, out_cb, kchunks=NCH,
                           ncht=NCH, wtag="w"):
                """out[:, cht] = win[i].T @ rhs (accumulate kchunks);
                rhs free width rhs_n per half; out_cb(cht, psum_tile)."""
                nhalf = rhs_n // 384
                for cht in range(ncht):
                    pts = ([ps.tile([128, 384], F32, tag="ps", name=f"bm{cht}_{x}")
                            for x in range(nhalf)] if nhalf > 1 else
                           [ps.tile([128, rhs_n], F32, tag="ps", name=f"bm{cht}")])
                    for ch in range(kchunks):
                        lw = scr.tile([128, 128], BF16, tag=wtag)
                        nc.sync.dma_start(
                            out=lw[:], in_=win[i, 128 * ch:128 * (ch + 1),
                                             128 * cht:128 * (cht + 1)])
                        for hf in range(len(pts)):
                            w = 384 if nhalf > 1 else rhs_n
                            nc.tensor.matmul(
                                pts[hf][:], lw[:],
                                rhs_tile[:, ch * rhs_n + hf * w:
                                         ch * rhs_n + hf * w + w],
                                start=(ch == 0), stop=(ch == kchunks - 1))
                    out_cb(cht, pts)

            # =================================================================
            # block loop
            # =================================================================
            for i in range(nblocks):
                tc.strict_bb_all_engine_barrier()
                sbi, j = divmod(i, SB)
                # ---- x assembly ----
                cs_s = load_cond(i, 0, "cnd_s")
                cs_t = load_cond(i, 1, "cnd_t")
                xT = scr1.tile([128, NCH * NS], BF16, tag="xT")
                lnT = scr1.tile([128, NCH * NS], BF16, tag="lnT")
                ln_chmajor(actT, lnT)
                for cht in range(NCH):
                    sl = slice(cht * NS, (cht + 1) * NS)
                    nc.vector.tensor_tensor(xT[:, sl], cs_s[:, sl], lnT[:, sl],
                                            ALU.mult)
                    nc.vector.tensor_tensor(xT[:, sl], xT[:, sl], cs_t[:, sl],
                                            ALU.add)
                # ---- AG(x) ----
                nc.sync.dma_start(
                    out=agx_in[i].ap().rearrange("(t p) s -> p t s", p=128),
                    in_=xT[:].rearrange("p (t s) -> p t s", s=NS))
                nc.gpsimd.collective_compute(
                    "AllGather", mybir.AluOpType.bypass, replica_groups=RG,
                    ins=[agx_in[i].ap().opt()], outs=[agx_out[i].ap().opt()])

                # ---- q, gate from local xT ----

                def q_out(cht, pts, _i=i):
                    qbp = scr.tile([128, 1], F32, tag="qbp")
                    nc.sync.dma_start(
                        out=qbp[:], in_=qb_in[_i, 128 * cht:128 * (cht + 1)])
                    nc.scalar.activation(qT[:, cht * NS:(cht + 1) * NS],
                                         pts[0][:], ACTF.Identity,
                                         bias=qbp[:])

                big_matmul(wq_in, i, xT, NS, q_out, ncht=NCP, wtag="wq")

                def g_out(cht, pts):
                    nc.scalar.activation(gsig[:, cht * NS:(cht + 1) * NS],
                                         pts[0][:], ACTF.Sigmoid)

                big_matmul(wg_in, i, xT, NS, g_out, ncht=NCP, wtag="wg")

                # ---- X readback ----
                for cht in range(NCH):
                    nc.sync.dma_start(
                        out=X[:].rearrange("p (t r s) -> p t r s", r=P,
                                           s=NS)[:, cht],
                        in_=agx_out[i].ap()
                            .rearrange("(r t p) s -> p t r s", p=128,
                                       r=P)[:, cht])

                # ---- k full (ch-major) ----
                def k_out(cht, pts):
                    for hf in range(2):
                        nc.vector.tensor_copy(
                            kT[:, cht * N + hf * 384:cht * N + (hf + 1) * 384],
                            pts[hf][:])

                big_matmul(wk_in, i, X, N, k_out, ncht=NCP, wtag="wk")

                # ---- v full (seq-major, X-stationary) + mask ----
                for half in range(2):
                    pvs = [ps.tile([128, 384], F32, tag="ps", name=f"pv{i}_{half}_{x}")
                           for x in range(NCH)]
                    for ch in range(NCH):
                        vw = scr.tile([128, 384], BF16, tag="wv")
                        nc.sync.dma_start(
                            out=vw[:],
                            in_=wv_in[i, 128 * ch:128 * (ch + 1),
                                      half * 384:(half + 1) * 384])
                        for st in range(NCH):
                            nc.tensor.matmul(
                                pvs[st][:],
                                X[:, ch * N + st * 128:ch * N + (st + 1) * 128],
                                vw[:], start=(ch == 0), stop=(ch == NCH - 1))
                    for st in range(NCH):
                        nc.vector.tensor_scalar(
                            vm[:, st * N + half * 384:st * N + (half + 1) * 384],
                            pvs[st][:], mask_sb[:, st:st + 1], None, ALU.mult)

                # ---- attention ----
                plt = plpool.tile([128, NH * NCH * NS], BF16, tag="plt")
                nc.sync.dma_start(
                    out=plt[:].rearrange("p (h c n) -> p h c n", c=NCH, n=NS),
                    in_=pl_dram[sbi, j * NH:(j + 1) * NH]
                        .rearrange("h (c p) n -> p h c n", p=128))
                for grp in range(2):
                    heads = list(range(grp * 8, grp * 8 + 8))
                    for ch in range(NCH):
                        for hh, h in enumerate(heads):
                            lg_ps = ps.tile([128, NS], F32, tag="ps")
                            hp, hr = h // 2, (h % 2) * 64
                            nc.tensor.matmul(
                                lg_ps[:],
                                kT[hr:hr + DH,
                                   hp * N + ch * 128:hp * N + (ch + 1) * 128],
                                qT[hr:hr + DH, hp * NS:(hp + 1) * NS],
                                start=True, stop=True)
                            lg = scr.tile([128, NS], BF16, tag="lg")
                            nc.vector.scalar_tensor_tensor(
                                lg[:], lg_ps[:], SCALE,
                                plt[:, (h * NCH + ch) * NS:
                                    (h * NCH + ch + 1) * NS],
                                ALU.mult, ALU.add)
                            nc.scalar.activation(
                                expb[:, (ch * 8 + hh) * NS:
                                     (ch * 8 + hh + 1) * NS],
                                lg[:], ACTF.Exp)
                    # denominators: s = mask.T @ exp  (all 8 heads at once)
                    for hf in range(2):
                        s_ps = ps.tile([1, 4 * NS], F32, tag="ps",
                                       name=f"sps{i}_{grp}_{hf}")
                        for st in range(NCH):
                            nc.tensor.matmul(
                                s_ps[:], mask_bf[:, st:st + 1],
                                expb[:, st * 8 * NS + hf * 4 * NS:
                                     st * 8 * NS + (hf + 1) * 4 * NS],
                                start=(st == 0), stop=(st == NCH - 1))
                        nc.vector.reciprocal(
                            rcp[:, (grp * 2 + hf) * 4 * NS:
                                (grp * 2 + hf + 1) * 4 * NS], s_ps[:])
                    for sub in range(2):      # 4 heads at a time (PSUM limit)
                        seg = grp * 2 + sub
                        sheads = heads[sub * 4:sub * 4 + 4]
                        wps = {}
                        for h in sheads:
                            hh = h - grp * 8
                            pw = ps.tile([48, NS], F32, tag="ps",
                                         name=f"pw{i}_{h}")
                            for ch in range(NCH):
                                nc.tensor.matmul(
                                    pw[:],
                                    vm[:, ch * N + h * DH:
                                       ch * N + (h + 1) * DH],
                                    expb[:, (ch * 8 + hh) * NS:
                                         (ch * 8 + hh + 1) * NS],
                                    start=(ch == 0), stop=(ch == NCH - 1))
                            wps[h] = pw
                        bps = ps.tile([48, 4 * NS], F32, tag="ps",
                                      name=f"bps{i}_{seg}")
                        nc.tensor.matmul(
                            bps[:], ones_row[:, 0:48],
                            rcp[:, seg * 4 * NS:(seg + 1) * 4 * NS],
                            start=True, stop=True)
                        nc.vector.tensor_copy(
                            bca[:, seg * 4 * NS:(seg + 1) * 4 * NS], bps[:])
                        for h in sheads:
                            hp, hr = h // 2, (h % 2) * 64
                            nc.vector.tensor_tensor(
                                waT[hr:hr + DH, hp * NS:(hp + 1) * NS],
                                wps[h][:], bca[:, h * NS:(h + 1) * NS],
                                ALU.mult)
                for cht in range(NCP):
                    sl = slice(cht * NS, (cht + 1) * NS)
                    nc.vector.tensor_tensor(waT[:, sl], waT[:, sl],
                                            gsig[:, sl], ALU.mult)

                # ---- u1 ----
                azc = load_cond(i, 2, "cnd_azc")

                def u1_out(cht, pts):
                    sl = slice(cht * NS, (cht + 1) * NS)
                    u1g = scr.tile([128, NS], F32, tag="u1g")
                    nc.vector.tensor_tensor(u1g[:], pts[0][:], azc[:, sl],
                                            ALU.mult)
                    nc.vector.tensor_tensor(actT[:, sl], actT[:, sl], u1g[:],
                                            ALU.add)

                big_matmul(wout_in, i, waT, NS, u1_out, kchunks=NCP,
                           wtag="wo")

                # ---- transition ----
                ct_s = load_cond(i, 3, "cnd_ts")
                ct_t = load_cond(i, 4, "cnd_tt")
                xtT = scr1.tile([128, NCH * NS], BF16, tag="xT")
                lnT2 = scr1.tile([128, NCH * NS], BF16, tag="lnT")
                ln_chmajor(actT, lnT2)
                for cht in range(NCH):
                    sl = slice(cht * NS, (cht + 1) * NS)
                    nc.vector.tensor_tensor(xtT[:, sl], ct_s[:, sl],
                                            lnT2[:, sl], ALU.mult)
                    nc.vector.tensor_tensor(xtT[:, sl], xtT[:, sl],
                                            ct_t[:, sl], ALU.add)
                nc.sync.dma_start(
                    out=agt_in[i].ap().rearrange("(t p) s -> p t s", p=128),
                    in_=xtT[:].rearrange("p (t s) -> p t s", s=NS))
                nc.gpsimd.collective_compute(
                    "AllGather", mybir.AluOpType.bypass, replica_groups=RG,
                    ins=[agt_in[i].ap().opt()], outs=[agt_out[i].ap().opt()])
                Xt = pp.tile([128, NCH * N], BF16, tag="X", bufs=2)
                for cht in range(NCH):
                    nc.sync.dma_start(
                        out=Xt[:].rearrange("p (t r s) -> p t r s", r=P,
                                            s=NS)[:, cht],
                        in_=agt_out[i].ap()
                            .rearrange("(r t p) s -> p t r s", p=128,
                                       r=P)[:, cht])

                csb = scr1.tile([128, 2 * N], BF16, tag="csb")
                for hc in range(2):
                    sa_h = {}
                    for ab in range(2):
                        col0 = ab * 256 + hc * 128
                        phs = [ps.tile([128, 384], F32, tag="ps", name=f"ph{i}_{hc}_{ab}_{x}")
                               for x in range(2)]
                        for ch in range(NCH):
                            lw = scr.tile([128, 128], BF16, tag="wt1")
                            nc.sync.dma_start(
                                out=lw[:],
                                in_=wtr1_in[i, 128 * ch:128 * (ch + 1),
                                            col0:col0 + 128])
                            for hf in range(2):
                                nc.tensor.matmul(
                                    phs[hf][:], lw[:],
                                    Xt[:, ch * N + hf * 384:
                                       ch * N + (hf + 1) * 384],
                                    start=(ch == 0), stop=(ch == NCH - 1))
                        bp = scr.tile([128, 1], F32, tag="bt1")
                        nc.sync.dma_start(out=bp[:],
                                          in_=btr1_in[i, col0:col0 + 128])
                        for hf in range(2):
                            if ab == 0:
                                sa = scr.tile([128, 384], BF16, tag="silua")
                                nc.scalar.activation(sa[:], phs[hf][:],
                                                     ACTF.Silu, bias=bp[:])
                                sa_h[hf] = sa
                            else:
                                bb = scr.tile([128, 384], F32, tag="hb")
                                nc.scalar.activation(bb[:], phs[hf][:],
                                                     ACTF.Identity, bias=bp[:])
                                nc.vector.tensor_tensor(
                                    csb[:, hc * N + hf * 384:
                                        hc * N + (hf + 1) * 384],
                                    sa_h[hf][:], bb[:], ALU.mult)

                def p2_out(cht, pts, _i=i):
                    for hf in range(2):
                        ev = scr.tile([128, 384], BF16, tag="p2ev")
                        nc.vector.tensor_copy(ev[:], pts[hf][:])
                        nc.sync.dma_start(
                            out=rs_in[_i][:, 128 * cht:128 * (cht + 1), :]
                                .rearrange("r c s -> c r s")[:, hf * 4:(hf + 1) * 4, :],
                            in_=ev[:].rearrange("c (r s) -> c r s", s=NS))

                big_matmul(wtr2_in, i, csb, N, p2_out, kchunks=2, wtag="wt2")
                nc.gpsimd.collective_compute(
                    "ReduceScatter", mybir.AluOpType.add, replica_groups=RG,
                    ins=[rs_in[i].ap().opt()], outs=[rs_out[i].ap().opt()])
                tazc = load_cond(i, 5, "cnd_tazc")
                u2 = scr1.tile([128, NCH * NS], BF16, tag="u2")
                nc.sync.dma_start(
                    out=u2[:].rearrange("p (t s) -> p t s", s=NS),
                    in_=rs_out[i].ap().rearrange("(t p) s -> p t s", p=128))
                for cht in range(NCH):
                    sl = slice(cht * NS, (cht + 1) * NS)
                    u2g = scr.tile([128, NS], F32, tag="u2g")
                    nc.vector.tensor_tensor(u2g[:], u2[:, sl], tazc[:, sl],
                                            ALU.mult)
                    nc.vector.tensor_tensor(actT[:, sl], actT[:, sl], u2g[:],
                                            ALU.add)

            nc.sync.dma_start(
                out=outT.ap().rearrange("(t p) s -> p t s", p=128),
                in_=actT[:].rearrange("p (t s) -> p t s", s=NS))

    nc.compile()
    return nc


# ------------------------------------------------------------------- runner
def _get_nc(nblocks=NB):
    if nblocks in _CACHE:
        return _CACHE[nblocks]
    nc = build_nc(nblocks=nblocks)
    _CACHE[nblocks] = nc
    return nc


def kernel(**inputs):
    import sys
    if "/opt/trn_rl_repo" not in sys.path:
        sys.path.insert(0, "/opt/trn_rl_repo")
    from concourse.bass_utils import run_bass_kernel_spmd
    nc = _get_nc()
    in_maps = host_prep(inputs)
    res = run_bass_kernel_spmd(nc, in_maps, core_ids=list(range(P)))
    out = np.zeros((N, CA), np.float32)
    for c in range(P):
        out[c * NS:(c + 1) * NS] = res.results[c]["outT"].T
    return out


if __name__ == "__main__":
    import reference
    inputs = reference.setup_inputs()
    out = kernel(**{k: np.asarray(v) for k, v in inputs.items()})
    print("out", out.shape, out.dtype)

